# revision 40
# baseline (speedup 1.0000x reference)
"""Trainium2 Bass kernel for conv-projected multi-head attention.

Reference computation (per batch item b of 8, one NeuronCore each):
  y   = BN(depthwise3x3(x_b reshaped to [C,32,32]))      # q = k = v = y
  q/k/v = y @ w{q,k,v}^T  (heads: 12 x 32)
  att = softmax((q @ k^T) * sqrt(32))
  out = (att @ v) @ wo^T

v5 design (ACT-exp-bound; bf16 everywhere so FWL hides all 128-col
LDWEIGHTS; batched DMAs; projections interleaved INTO the attention
stream so the first exp fires right after the conv):
 - conv: 9 accumulating diag-matmuls on PE over a zero-padded [34x34] image
 - qT/kT [o, t] bf16 via lhsT=w^T; v stored [t, h, d] bf16 (vsb)
 - scores/exp in 8 groups of 3 heads (sg in 4, lh in 2 query halves):
     3 row-tiled (K=32) matmuls -> s4 psum [128, 1536] (3 banks, double
     buffered so the ACT exp never waits on a WAR hazard); exp on ACT
     psum->SBUF bf16 with scale=sqrt(32) fused.
 - AV/sums in 6 groups of 4 heads, all 4 col strips:
     AV: 4 col-tiled (M=32) matmuls, lhsT=v_h, each streaming its own E_h,
       accumulated over the 8 t-tiles into one psum bank (rows 32j)
     sums: 4 col-tiled matmuls with lhsT=ones[128,32] -> softmax denoms
     normalize: DVE fast-reciprocal + one [128,512] tensor_tensor mult
       -> attn c-tiles [128, T] bf16
   AV lags 2 score-groups in the lh=0 half (so the shared psum tag is
   free for the interleaved projections) and 1 group in the lh=1 half.
 - out projection: lh=0 half is emitted inside lh=1's first score group
   (a window with no AV work); lh=1 at the end.
PSUM: tag s4 2x3 banks + tag ps512 2x1 banks (conv/proj/ov/sm/outproj
ring) = 8 banks.
"""
import sys

sys.path.insert(0, "/opt/trn_rl_repo")
from contextlib import ExitStack

import numpy as np

B, T, C = 8, 1024, 384
NH, DH = 12, 32
HH = WW = 32
SCALE = float(DH) ** 0.5
BN_EPS = 1e-5
NCORES = 8
SG, HPS = 4, 3  # score groups: 4 groups of 3 heads
AG, HPA = 3, 4  # AV groups: 3 groups of 4 heads

_CACHE = {}


def _build(debug=False):
    import concourse.bass as bass
    import concourse.tile as tile
    from concourse import bacc, mybir
    from concourse.masks import make_identity
    from concourse.dve_ops import RECIPROCAL_APPROX_FAST, RECIP_APPROX_FAST_CONSTS

    F32 = mybir.dt.float32
    BF16 = mybir.dt.bfloat16
    AF = mybir.ActivationFunctionType
    ALU = mybir.AluOpType

    nc = bacc.Bacc("TRN2", target_bir_lowering=False, debug=False)

    xt_d = nc.dram_tensor("xt", [C, T], BF16, kind="ExternalInput").ap()
    w9_d = nc.dram_tensor("w9", [C, 9], F32, kind="ExternalInput").ap()
    bias_d = nc.dram_tensor("bias", [C, 1], F32, kind="ExternalInput").ap()
    wqT_d = nc.dram_tensor("wqT", [C, C], BF16, kind="ExternalInput").ap()
    wkT_d = nc.dram_tensor("wkT", [C, C], BF16, kind="ExternalInput").ap()
    wvT_d = nc.dram_tensor("wvT", [C, C], BF16, kind="ExternalInput").ap()
    woT_d = nc.dram_tensor("woT", [C, C], BF16, kind="ExternalInput").ap()
    ones_d = nc.dram_tensor("ones32", [128, 32], F32, kind="ExternalInput").ap()
    outT_d = nc.dram_tensor("outT", [C, T], BF16, kind="ExternalOutput").ap()

    CT = C // 128  # 3 c-tiles
    TT = T // 128  # 8 t-tiles
    TH = T // 512  # 2 t-halves / l-halves

    with tile.TileContext(nc) as tc, ExitStack() as top:
        persist = top.enter_context(tc.tile_pool(name="persist", bufs=1))
        copies = top.enter_context(tc.tile_pool(name="copies", bufs=3))
        psum = top.enter_context(tc.tile_pool(name="psum", bufs=2, space="PSUM"))
        epool = top.enter_context(tc.tile_pool(name="epool", bufs=32))
        rpool = top.enter_context(tc.tile_pool(name="rpool", bufs=2))
        setup = top.enter_context(tc.tile_pool(name="setup", bufs=1))

        y_sb = [persist.tile([128, T], BF16, tag=f"y{i}", name=f"y{i}") for i in range(CT)]
        qT_sb = [persist.tile([128, T], BF16, tag=f"q{i}", name=f"q{i}") for i in range(CT)]
        kT_sb = [persist.tile([128, T], BF16, tag=f"k{i}", name=f"k{i}") for i in range(CT)]
        vsb = [persist.tile([128, NH, DH], BF16, tag=f"v{i}", name=f"v{i}") for i in range(TT)]
        attn_sb = [persist.tile([128, T], BF16, tag=f"at{i}", name=f"at{i}") for i in range(CT)]
        ones_bf = persist.tile([128, DH], BF16, tag="ones", name="ones")
        wsb = {}
        for nm in ("k", "q", "v", "o"):
            wsb[nm] = persist.tile([128, CT, C], BF16, tag=f"w{nm}", name=f"w{nm}")

        def ps512(name):
            return psum.tile([128, 512], F32, tag="ps512", name=name)

        # ---- all input DMAs first, in consumption order: w9 (diag builds
        # don't wait on xt), xt per c-tile (plain 2D transfers — the
        # rearranged 3D pattern measured ~2.5x slower), then weights ----
        w9_sb = setup.tile([128, CT, 9], F32, tag="w9b", name="w9b")
        nc.sync.dma_start(w9_sb[:], w9_d.rearrange("(i p) n -> p i n", p=128))
        xtb = setup.tile([128, CT, T], BF16, tag="xtb", name="xtb")
        for i in range(CT):
            nc.sync.dma_start(xtb[:, i, :], xt_d[i * 128 : (i + 1) * 128, :])
        for nm, d in (("k", wkT_d), ("q", wqT_d)):
            nc.sync.dma_start(wsb[nm][:], d.rearrange("(i p) n -> p i n", p=128))
        bias_sb = setup.tile([128, CT], F32, tag="biasb", name="biasb")
        nc.sync.dma_start(bias_sb[:], bias_d.rearrange("(i p) n -> p (i n)", p=128))
        ones_f32 = setup.tile([128, DH], F32, tag="ones_f")
        nc.sync.dma_start(ones_f32[:], ones_d)
        for nm, d in (("v", wvT_d), ("o", woT_d)):
            nc.sync.dma_start(wsb[nm][:], d.rearrange("(i p) n -> p i n", p=128))

        # ---- padded input and diag weights ----
        xp = [setup.tile([128, 34 * 34], BF16, tag=f"xp{i}", name=f"xp{i}") for i in range(CT)]
        ident = setup.tile([128, 128], F32, tag="ident")
        identb = setup.tile([128, 128], BF16, tag="identb")
        diag = [setup.tile([128, 9, 128], BF16, tag=f"dg{i}", name=f"dg{i}") for i in range(CT)]

        make_identity(nc, ident[:])
        nc.vector.tensor_copy(identb[:], ident[:])
        # HAM warm-up: keep the PE streaming during the DMA window so the
        # clock gate is at 8/8 when the conv starts (results unused; the
        # first real s4-ring user overwrites with start=True)
        wt = None
        for w in range(40):
            if w % 20 == 0:
                wt = psum.tile([128, 512], F32, tag="s4", name="wt")
            nc.tensor.matmul(
                wt[:, 0:128], identb[:], identb[:], start=True, stop=True
            )
        for i in range(CT):
            nc.vector.memset(xp[i][:].bitcast(F32), 0.0)
        # diag builds before the image copies: they wait only on the tiny
        # w9 DMA, so the DVE queue isn't parked behind the xt transfer
        for i in range(CT):
            for k in range(9):
                nc.vector.tensor_scalar_mul(
                    diag[i][:, k, :], ident[:], w9_sb[:, i, k : k + 1]
                )
            nc.vector.tensor_copy(
                xp[i][:].rearrange("p (a b) -> p a b", a=34)[:, 1:33, 1:33],
                xtb[:, i, :].rearrange("p (a b) -> p a b", a=32),
            )
        # preload the exp table set on ACT while conv/DMA run
        warm = setup.tile([128, 8], F32, tag="warm")
        nc.vector.memset(warm[:], 0.0)
        nc.scalar.activation(warm[:], warm[:], AF.Exp)
        nc.vector.tensor_copy(ones_bf[:], ones_f32[:])

        # ---- conv emitter: 9 accumulating diag matmuls per (c-tile,
        # t-half); emitted t-half-outer from the schedule so the first
        # score group starts after only half the conv ----
        def conv_tile(i, th):
            yp = ps512("yp")
            r0 = th * 16
            for k in range(9):
                dy, dx = k // 3 - 1, k % 3 - 1
                off = (r0 + 1 + dy) * 34 + (1 + dx)
                rhs = bass.AP(
                    tensor=xp[i].tensor,
                    offset=xp[i].offset + off,
                    ap=[list(p) for p in xp[i].ap[:1]] + [[34, 16], [1, 32]],
                )
                nc.tensor.matmul(
                    yp[:].rearrange("p (a b) -> p a b", a=16),
                    diag[i][:, k, :],
                    rhs,
                    start=(k == 0),
                    stop=(k == 8),
                )
            nc.vector.tensor_scalar_add(
                y_sb[i][:, th * 512 : (th + 1) * 512],
                yp[:],
                bias_sb[:, i : i + 1],
            )

        # ---- projection emitters (interleaved into the attention stream) --
        def proj_qk(nm, dst, ot, ths):
            for th in ths:
                pp = ps512("pp")
                for kt in range(CT):
                    nc.tensor.matmul(
                        pp[:],
                        wsb[nm][:, kt, ot * 128 : (ot + 1) * 128],
                        y_sb[kt][:, th * 512 : (th + 1) * 512],
                        start=(kt == 0),
                        stop=(kt == CT - 1),
                    )
                nc.vector.tensor_copy(dst[ot][:, th * 512 : (th + 1) * 512], pp[:])

        def proj_v():
            for tt in range(TT):
                vp = ps512("vp")
                for kt in range(CT):
                    nc.tensor.matmul(
                        vp[:, 0:C],
                        y_sb[kt][:, tt * 128 : (tt + 1) * 128],
                        wsb["v"][:, kt, :],
                        start=(kt == 0),
                        stop=(kt == CT - 1),
                    )
                nc.vector.tensor_copy(
                    vsb[tt][:], vp[:, 0:C].rearrange("p (h d) -> p h d", h=NH)
                )

        def out_proj(th, use_s4=False):
            for ot in range(CT):
                if use_s4:
                    # after the last exp the scores ring is free; avoids
                    # serializing through the held ov/sm ps512 slots
                    op = psum.tile([128, 512], F32, tag="s4", name="op")
                else:
                    op = ps512("op")
                for kt in range(CT):
                    nc.tensor.matmul(
                        op[:],
                        wsb["o"][:, kt, ot * 128 : (ot + 1) * 128],
                        attn_sb[kt][:, th * 512 : (th + 1) * 512],
                        start=(kt == 0),
                        stop=(kt == CT - 1),
                    )
                oc = copies.tile([128, 512], BF16, tag="oc")
                nc.vector.tensor_copy(oc[:], op[:])
                nc.sync.dma_start(
                    outT_d[ot * 128 : (ot + 1) * 128, th * 512 : (th + 1) * 512],
                    oc[:],
                )

        # ---- attention emitters ----
        # av_state[(lh, a)] = (ov, sm, E_store); E stores are per (lh).
        state = {"E": None, "av": {}, "lh": 0}

        def score_tile(g, tt):
            lh = state["lh"]
            s4 = psum.tile([128, HPS * 512], F32, tag="s4", name="s4")
            for j in range(HPS):
                h = HPS * g + j
                ct, r = h // 4, 32 * (h % 4)
                nc.tensor.matmul(
                    s4[:, 512 * j : 512 * (j + 1)],
                    kT_sb[ct][r : r + 32, tt * 128 : (tt + 1) * 128],
                    qT_sb[ct][r : r + 32, lh * 512 : (lh + 1) * 512],
                    start=True,
                    stop=True,
                    tile_position=(r, 0),
                )
            e = epool.tile([128, HPS * 512], BF16, tag="E", name="e")
            nc.scalar.activation(e[:], s4[:], AF.Exp, scale=SCALE)
            state["E"][g][tt] = e

        def scores_group(g, av_plan=None, av_tail=None, deferred_tail=False):
            """Emit score group g (3 heads) for current lh; av_plan is an
            optional (key, lag) pair to interleave AV/sums matmuls of the
            AV group registered under key (possibly from the other lh).
            av_tail finishes a previous group's last av_sums + normalize
            inside this group's pipeline (instead of stalling the exp
            stream at the boundary); deferred_tail leaves this group's own
            tail for a later av_tail."""
            for tt in range(TT):
                score_tile(g, tt)
                if av_tail is not None and tt == 1:
                    for t2 in range(TT - av_tail[1], TT):
                        av_sums(av_tail[0], t2)
                    normalize(av_tail[0])
                if av_plan is not None:
                    key, lag = av_plan
                    if tt >= lag:
                        av_sums(key, tt - lag)
            if av_plan is not None and not deferred_tail:
                key, lag = av_plan
                for tt in range(TT - lag, TT):
                    av_sums(key, tt)
                normalize(key)

        def av_alloc(key, E_store):
            ovt = ps512("ovt")
            smt = ps512("smt")
            state["av"][key] = (ovt, smt, E_store)

        def av_sums(key, tt):
            ov, sm, E_store = state["av"][key]
            a = key[1]
            for j in range(HPA):
                h = HPA * a + j
                g, jj = h // HPS, h % HPS
                es = E_store[g][tt][:, 512 * jj : 512 * (jj + 1)]
                nc.tensor.matmul(
                    ov[32 * j : 32 * (j + 1), :],
                    vsb[tt][:, h, :],
                    es,
                    start=(tt == 0),
                    stop=(tt == TT - 1),
                    tile_position=(0, 32 * j),
                )
            for j in range(HPA):
                h = HPA * a + j
                g, jj = h // HPS, h % HPS
                es = E_store[g][tt][:, 512 * jj : 512 * (jj + 1)]
                nc.tensor.matmul(
                    sm[32 * j : 32 * (j + 1), :],
                    ones_bf[:],
                    es,
                    start=(tt == 0),
                    stop=(tt == TT - 1),
                    tile_position=(0, 32 * j),
                )

        def normalize(key):
            lh, a = key
            ov, sm, _ = state["av"][key]
            rr = rpool.tile([128, 512], F32, tag="rr", name="rr")
            nc.vector._custom_dve(
                RECIPROCAL_APPROX_FAST,
                out=rr[:],
                in0=sm[:],
                s0=RECIP_APPROX_FAST_CONSTS["s0"],
                s1=RECIP_APPROX_FAST_CONSTS["s1"],
                imm2=RECIP_APPROX_FAST_CONSTS["imm2"],
            )
            nc.vector.tensor_tensor(
                attn_sb[a][:, lh * 512 : (lh + 1) * 512],
                ov[:],
                rr[:],
                ALU.mult,
            )

        # ---- the schedule ----
        # lh = 0: projections interleaved between score groups, AV lagging
        # two groups so the shared ps512 ring is projection-free by then.
        E0 = [[None] * TT for _ in range(SG)]
        state["lh"], state["E"] = 0, E0
        # conv t-half 0, then the ot=0 projections of that half, then the
        # first score tiles — with conv t-half 1 interleaved between them
        for i in range(CT):
            conv_tile(i, 0)
        proj_qk("k", kT_sb, 0, [0])
        proj_qk("q", qT_sb, 0, [0])
        score_tile(0, 0)
        conv_tile(0, 1)
        score_tile(0, 1)
        conv_tile(1, 1)
        score_tile(0, 2)
        conv_tile(2, 1)
        score_tile(0, 3)
        proj_qk("k", kT_sb, 0, [1])
        for tt in range(4, TT):
            score_tile(0, tt)
        proj_qk("k", kT_sb, 1, range(TH))
        proj_qk("q", qT_sb, 1, [0])
        scores_group(1)
        proj_qk("k", kT_sb, 2, range(TH))
        proj_qk("q", qT_sb, 2, [0])
        proj_v()
        for ot in range(CT):
            proj_qk("q", qT_sb, ot, [1])
        av_alloc((0, 0), E0)
        scores_group(2, av_plan=((0, 0), 0))
        av_alloc((0, 1), E0)
        scores_group(3, av_plan=((0, 1), 0))

        # lh = 1: lh0's trailing AV group and out_proj(0) fill the AV-free
        # windows of the first two score groups.
        E1 = [[None] * TT for _ in range(SG)]
        state["lh"], state["E"] = 1, E1
        av_alloc((0, 2), E0)
        scores_group(0, av_plan=((0, 2), 0))
        av_alloc((1, 0), E1)
        scores_group(1, av_plan=((1, 0), 2), deferred_tail=True)
        av_alloc((1, 1), E1)
        scores_group(2, av_plan=((1, 1), 2), deferred_tail=True,
                     av_tail=((1, 0), 2))
        # out_proj(0) here: its ring WARs (normalize/recip of av group
        # (1,0)) fired during g2, so the 12 matmuls run stall-free in
        # g2's tail window instead of lengthening the lh-boundary chain
        out_proj(0)
        av_alloc((1, 2), E1)
        scores_group(3, av_plan=((1, 2), 2), deferred_tail=True,
                     av_tail=((1, 1), 2))
        for tt in range(TT - 2, TT):
            av_sums((1, 2), tt)
        normalize((1, 2))
        out_proj(1, use_s4=True)

    nc.compile()
    return nc


def _prep_inputs(x, conv_w, bn_gamma, bn_beta, bn_mean, bn_var, wq, wk, wv, wo):
    import ml_dtypes

    f32 = np.float32
    bf16 = ml_dtypes.bfloat16
    inv = (bn_gamma / np.sqrt(bn_var + BN_EPS)).astype(f32)
    w9 = (conv_w.reshape(C, 9) * inv[:, None]).astype(f32)
    bias = (bn_beta - bn_mean * inv).astype(f32).reshape(C, 1)
    wqT = np.ascontiguousarray(np.asarray(wq, f32).T).astype(bf16)
    wkT = np.ascontiguousarray(np.asarray(wk, f32).T).astype(bf16)
    wvT = np.ascontiguousarray(np.asarray(wv, f32).T).astype(bf16)
    woT = np.ascontiguousarray(np.asarray(wo, f32).T).astype(bf16)
    ones32 = np.ones((128, 32), f32)
    maps = []
    for b in range(B):
        maps.append(
            {
                "xt": np.ascontiguousarray(np.asarray(x[b], f32).T).astype(bf16),
                "w9": w9,
                "bias": bias,
                "wqT": wqT,
                "wkT": wkT,
                "wvT": wvT,
                "woT": woT,
                "ones32": ones32,
            }
        )
    return maps


def kernel(x, conv_w, bn_gamma, bn_beta, bn_mean, bn_var, wq, wk, wv, wo, h, w,
           **kw):
    assert int(h) == HH and int(w) == WW
    from concourse.bass_utils import run_bass_kernel_spmd

    if "nc" not in _CACHE:
        _CACHE["nc"] = _build()
    nc = _CACHE["nc"]
    maps = _prep_inputs(
        x, conv_w, bn_gamma, bn_beta, bn_mean, bn_var, wq, wk, wv, wo
    )
    res = run_bass_kernel_spmd(nc, maps, list(range(NCORES)))
    out = np.stack([res.results[b]["outT"].T for b in range(B)])
    return out.astype(np.float32)


# revision 41
# speedup vs baseline: 1.0017x; 1.0017x over previous
"""Trainium2 Bass kernel for conv-projected multi-head attention.

Reference computation (per batch item b of 8, one NeuronCore each):
  y   = BN(depthwise3x3(x_b reshaped to [C,32,32]))      # q = k = v = y
  q/k/v = y @ w{q,k,v}^T  (heads: 12 x 32)
  att = softmax((q @ k^T) * sqrt(32))
  out = (att @ v) @ wo^T

v5 design (ACT-exp-bound; bf16 everywhere so FWL hides all 128-col
LDWEIGHTS; batched DMAs; projections interleaved INTO the attention
stream so the first exp fires right after the conv):
 - conv: 9 accumulating diag-matmuls on PE over a zero-padded [34x34] image
 - qT/kT [o, t] bf16 via lhsT=w^T; v stored [t, h, d] bf16 (vsb)
 - scores/exp in 8 groups of 3 heads (sg in 4, lh in 2 query halves):
     3 row-tiled (K=32) matmuls -> s4 psum [128, 1536] (3 banks, double
     buffered so the ACT exp never waits on a WAR hazard); exp on ACT
     psum->SBUF bf16 with scale=sqrt(32) fused.
 - AV/sums in 6 groups of 4 heads, all 4 col strips:
     AV: 4 col-tiled (M=32) matmuls, lhsT=v_h, each streaming its own E_h,
       accumulated over the 8 t-tiles into one psum bank (rows 32j)
     sums: 4 col-tiled matmuls with lhsT=ones[128,32] -> softmax denoms
     normalize: DVE fast-reciprocal + one [128,512] tensor_tensor mult
       -> attn c-tiles [128, T] bf16
   AV lags 2 score-groups in the lh=0 half (so the shared psum tag is
   free for the interleaved projections) and 1 group in the lh=1 half.
 - out projection: lh=0 half is emitted inside lh=1's first score group
   (a window with no AV work); lh=1 at the end.
PSUM: tag s4 2x3 banks + tag ps512 2x1 banks (conv/proj/ov/sm/outproj
ring) = 8 banks.
"""
import sys

sys.path.insert(0, "/opt/trn_rl_repo")
from contextlib import ExitStack

import numpy as np

B, T, C = 8, 1024, 384
NH, DH = 12, 32
HH = WW = 32
SCALE = float(DH) ** 0.5
BN_EPS = 1e-5
NCORES = 8
SG, HPS = 4, 3  # score groups: 4 groups of 3 heads
AG, HPA = 3, 4  # AV groups: 3 groups of 4 heads

_CACHE = {}


def _build(debug=False):
    import concourse.bass as bass
    import concourse.tile as tile
    from concourse import bacc, mybir
    from concourse.masks import make_identity
    from concourse.dve_ops import RECIPROCAL_APPROX_FAST, RECIP_APPROX_FAST_CONSTS

    F32 = mybir.dt.float32
    BF16 = mybir.dt.bfloat16
    AF = mybir.ActivationFunctionType
    ALU = mybir.AluOpType

    nc = bacc.Bacc("TRN2", target_bir_lowering=False, debug=False)

    xt_d = nc.dram_tensor("xt", [C, T], BF16, kind="ExternalInput").ap()
    w9_d = nc.dram_tensor("w9", [C, 9], F32, kind="ExternalInput").ap()
    bias_d = nc.dram_tensor("bias", [C, 1], F32, kind="ExternalInput").ap()
    wqT_d = nc.dram_tensor("wqT", [C, C], BF16, kind="ExternalInput").ap()
    wkT_d = nc.dram_tensor("wkT", [C, C], BF16, kind="ExternalInput").ap()
    wvT_d = nc.dram_tensor("wvT", [C, C], BF16, kind="ExternalInput").ap()
    woT_d = nc.dram_tensor("woT", [C, C], BF16, kind="ExternalInput").ap()
    ones_d = nc.dram_tensor("ones32", [128, 32], F32, kind="ExternalInput").ap()
    outT_d = nc.dram_tensor("outT", [C, T], BF16, kind="ExternalOutput").ap()

    CT = C // 128  # 3 c-tiles
    TT = T // 128  # 8 t-tiles
    TH = T // 512  # 2 t-halves / l-halves

    with tile.TileContext(nc) as tc, ExitStack() as top:
        persist = top.enter_context(tc.tile_pool(name="persist", bufs=1))
        copies = top.enter_context(tc.tile_pool(name="copies", bufs=3))
        psum = top.enter_context(tc.tile_pool(name="psum", bufs=2, space="PSUM"))
        epool = top.enter_context(tc.tile_pool(name="epool", bufs=32))
        rpool = top.enter_context(tc.tile_pool(name="rpool", bufs=2))
        setup = top.enter_context(tc.tile_pool(name="setup", bufs=1))

        y_sb = [persist.tile([128, T], BF16, tag=f"y{i}", name=f"y{i}") for i in range(CT)]
        qT_sb = [persist.tile([128, T], BF16, tag=f"q{i}", name=f"q{i}") for i in range(CT)]
        kT_sb = [persist.tile([128, T], BF16, tag=f"k{i}", name=f"k{i}") for i in range(CT)]
        vsb = [persist.tile([128, NH, DH], BF16, tag=f"v{i}", name=f"v{i}") for i in range(TT)]
        attn_sb = [persist.tile([128, T], BF16, tag=f"at{i}", name=f"at{i}") for i in range(CT)]
        ones_bf = persist.tile([128, DH], BF16, tag="ones", name="ones")
        wsb = {}
        for nm in ("k", "q", "v", "o"):
            wsb[nm] = persist.tile([128, CT, C], BF16, tag=f"w{nm}", name=f"w{nm}")

        def ps512(name):
            return psum.tile([128, 512], F32, tag="ps512", name=name)

        # ---- all input DMAs first, in consumption order: w9 (diag builds
        # don't wait on xt), xt per c-tile (plain 2D transfers — the
        # rearranged 3D pattern measured ~2.5x slower), then weights ----
        w9_sb = setup.tile([128, CT, 9], F32, tag="w9b", name="w9b")
        nc.sync.dma_start(w9_sb[:], w9_d.rearrange("(i p) n -> p i n", p=128))
        xtb = setup.tile([128, CT, T], BF16, tag="xtb", name="xtb")
        for i in range(CT):
            nc.sync.dma_start(xtb[:, i, :], xt_d[i * 128 : (i + 1) * 128, :])
        for nm, d in (("k", wkT_d), ("q", wqT_d)):
            nc.sync.dma_start(wsb[nm][:], d.rearrange("(i p) n -> p i n", p=128))
        bias_sb = setup.tile([128, CT], F32, tag="biasb", name="biasb")
        nc.sync.dma_start(bias_sb[:], bias_d.rearrange("(i p) n -> p (i n)", p=128))
        ones_f32 = setup.tile([128, DH], F32, tag="ones_f")
        nc.sync.dma_start(ones_f32[:], ones_d)
        for nm, d in (("v", wvT_d), ("o", woT_d)):
            nc.sync.dma_start(wsb[nm][:], d.rearrange("(i p) n -> p i n", p=128))

        # ---- padded input and diag weights ----
        xp = [setup.tile([128, 34 * 34], BF16, tag=f"xp{i}", name=f"xp{i}") for i in range(CT)]
        ident = setup.tile([128, 128], F32, tag="ident")
        identb = setup.tile([128, 128], BF16, tag="identb")
        diag = [setup.tile([128, 9, 128], BF16, tag=f"dg{i}", name=f"dg{i}") for i in range(CT)]

        make_identity(nc, ident[:])
        nc.vector.tensor_copy(identb[:], ident[:])
        # HAM warm-up: keep the PE streaming during the DMA window so the
        # clock gate is at 8/8 when the conv starts (results unused; the
        # first real s4-ring user overwrites with start=True)
        wt = None
        for w in range(40):
            if w % 20 == 0:
                wt = psum.tile([128, 512], F32, tag="s4", name="wt")
            nc.tensor.matmul(
                wt[:, 0:128], identb[:], identb[:], start=True, stop=True
            )
        for i in range(CT):
            nc.vector.memset(xp[i][:].bitcast(F32), 0.0)
        # diag builds before the image copies: they wait only on the tiny
        # w9 DMA, so the DVE queue isn't parked behind the xt transfer
        for i in range(CT):
            for k in range(9):
                nc.vector.tensor_scalar_mul(
                    diag[i][:, k, :], ident[:], w9_sb[:, i, k : k + 1]
                )
            nc.vector.tensor_copy(
                xp[i][:].rearrange("p (a b) -> p a b", a=34)[:, 1:33, 1:33],
                xtb[:, i, :].rearrange("p (a b) -> p a b", a=32),
            )
        # preload the exp table set on ACT while conv/DMA run
        warm = setup.tile([128, 8], F32, tag="warm")
        nc.vector.memset(warm[:], 0.0)
        nc.scalar.activation(warm[:], warm[:], AF.Exp)
        nc.vector.tensor_copy(ones_bf[:], ones_f32[:])

        # ---- conv emitter: 9 accumulating diag matmuls per (c-tile,
        # t-half); emitted t-half-outer from the schedule so the first
        # score group starts after only half the conv ----
        def conv_tile(i, th):
            yp = ps512("yp")
            r0 = th * 16
            for k in range(9):
                dy, dx = k // 3 - 1, k % 3 - 1
                off = (r0 + 1 + dy) * 34 + (1 + dx)
                rhs = bass.AP(
                    tensor=xp[i].tensor,
                    offset=xp[i].offset + off,
                    ap=[list(p) for p in xp[i].ap[:1]] + [[34, 16], [1, 32]],
                )
                nc.tensor.matmul(
                    yp[:].rearrange("p (a b) -> p a b", a=16),
                    diag[i][:, k, :],
                    rhs,
                    start=(k == 0),
                    stop=(k == 8),
                )
            nc.vector.tensor_scalar_add(
                y_sb[i][:, th * 512 : (th + 1) * 512],
                yp[:],
                bias_sb[:, i : i + 1],
            )

        # ---- projection emitters (interleaved into the attention stream) --
        def proj_qk(nm, dst, ot, ths):
            for th in ths:
                pp = ps512("pp")
                for kt in range(CT):
                    nc.tensor.matmul(
                        pp[:],
                        wsb[nm][:, kt, ot * 128 : (ot + 1) * 128],
                        y_sb[kt][:, th * 512 : (th + 1) * 512],
                        start=(kt == 0),
                        stop=(kt == CT - 1),
                    )
                nc.vector.tensor_copy(dst[ot][:, th * 512 : (th + 1) * 512], pp[:])

        def proj_v():
            for tt in range(TT):
                vp = ps512("vp")
                for kt in range(CT):
                    nc.tensor.matmul(
                        vp[:, 0:C],
                        y_sb[kt][:, tt * 128 : (tt + 1) * 128],
                        wsb["v"][:, kt, :],
                        start=(kt == 0),
                        stop=(kt == CT - 1),
                    )
                nc.vector.tensor_copy(
                    vsb[tt][:], vp[:, 0:C].rearrange("p (h d) -> p h d", h=NH)
                )

        def out_proj(th, use_s4=False):
            for ot in range(CT):
                if use_s4:
                    # after the last exp the scores ring is free; avoids
                    # serializing through the held ov/sm ps512 slots
                    op = psum.tile([128, 512], F32, tag="s4", name="op")
                else:
                    op = ps512("op")
                for kt in range(CT):
                    nc.tensor.matmul(
                        op[:],
                        wsb["o"][:, kt, ot * 128 : (ot + 1) * 128],
                        attn_sb[kt][:, th * 512 : (th + 1) * 512],
                        start=(kt == 0),
                        stop=(kt == CT - 1),
                    )
                oc = copies.tile([128, 512], BF16, tag="oc")
                nc.vector.tensor_copy(oc[:], op[:])
                nc.sync.dma_start(
                    outT_d[ot * 128 : (ot + 1) * 128, th * 512 : (th + 1) * 512],
                    oc[:],
                )

        # ---- attention emitters ----
        # av_state[(lh, a)] = (ov, sm, E_store); E stores are per (lh).
        state = {"E": None, "av": {}, "lh": 0}

        def score_tile(g, tt):
            lh = state["lh"]
            s4 = psum.tile([128, HPS * 512], F32, tag="s4", name="s4")
            for j in range(HPS):
                h = HPS * g + j
                ct, r = h // 4, 32 * (h % 4)
                nc.tensor.matmul(
                    s4[:, 512 * j : 512 * (j + 1)],
                    kT_sb[ct][r : r + 32, tt * 128 : (tt + 1) * 128],
                    qT_sb[ct][r : r + 32, lh * 512 : (lh + 1) * 512],
                    start=True,
                    stop=True,
                    tile_position=(r, 0),
                )
            e = epool.tile([128, HPS * 512], BF16, tag="E", name="e")
            nc.scalar.activation(e[:], s4[:], AF.Exp, scale=SCALE)
            state["E"][g][tt] = e

        def scores_group(g, av_plan=None, av_tail=None, deferred_tail=False):
            """Emit score group g (3 heads) for current lh; av_plan is an
            optional (key, lag) pair to interleave AV/sums matmuls of the
            AV group registered under key (possibly from the other lh).
            av_tail finishes a previous group's last av_sums + normalize
            inside this group's pipeline (instead of stalling the exp
            stream at the boundary); deferred_tail leaves this group's own
            tail for a later av_tail."""
            for tt in range(TT):
                score_tile(g, tt)
                if av_tail is not None and tt == 1:
                    for t2 in range(TT - av_tail[1], TT):
                        av_sums(av_tail[0], t2)
                    normalize(av_tail[0])
                if av_plan is not None:
                    key, lag = av_plan
                    if tt >= lag:
                        av_sums(key, tt - lag)
            if av_plan is not None and not deferred_tail:
                key, lag = av_plan
                for tt in range(TT - lag, TT):
                    av_sums(key, tt)
                normalize(key)

        def av_alloc(key, E_store):
            ovt = ps512("ovt")
            smt = ps512("smt")
            state["av"][key] = (ovt, smt, E_store)

        def av_sums(key, tt):
            ov, sm, E_store = state["av"][key]
            a = key[1]
            for j in range(HPA):
                h = HPA * a + j
                g, jj = h // HPS, h % HPS
                es = E_store[g][tt][:, 512 * jj : 512 * (jj + 1)]
                nc.tensor.matmul(
                    ov[32 * j : 32 * (j + 1), :],
                    vsb[tt][:, h, :],
                    es,
                    start=(tt == 0),
                    stop=(tt == TT - 1),
                    tile_position=(0, 32 * j),
                )
            for j in range(HPA):
                h = HPA * a + j
                g, jj = h // HPS, h % HPS
                es = E_store[g][tt][:, 512 * jj : 512 * (jj + 1)]
                nc.tensor.matmul(
                    sm[32 * j : 32 * (j + 1), :],
                    ones_bf[:],
                    es,
                    start=(tt == 0),
                    stop=(tt == TT - 1),
                    tile_position=(0, 32 * j),
                )

        def normalize(key):
            lh, a = key
            ov, sm, _ = state["av"][key]
            rr = rpool.tile([128, 512], F32, tag="rr", name="rr")
            nc.vector._custom_dve(
                RECIPROCAL_APPROX_FAST,
                out=rr[:],
                in0=sm[:],
                s0=RECIP_APPROX_FAST_CONSTS["s0"],
                s1=RECIP_APPROX_FAST_CONSTS["s1"],
                imm2=RECIP_APPROX_FAST_CONSTS["imm2"],
            )
            nc.vector.tensor_tensor(
                attn_sb[a][:, lh * 512 : (lh + 1) * 512],
                ov[:],
                rr[:],
                ALU.mult,
            )

        # ---- the schedule ----
        # lh = 0: projections interleaved between score groups, AV lagging
        # two groups so the shared ps512 ring is projection-free by then.
        E0 = [[None] * TT for _ in range(SG)]
        state["lh"], state["E"] = 0, E0
        # conv t-half 0, then the ot=0 projections of that half, then the
        # first score tiles — with conv t-half 1 interleaved between them
        for i in range(CT):
            conv_tile(i, 0)
        proj_qk("k", kT_sb, 0, [0])
        proj_qk("q", qT_sb, 0, [0])
        score_tile(0, 0)
        conv_tile(0, 1)
        score_tile(0, 1)
        conv_tile(1, 1)
        score_tile(0, 2)
        conv_tile(2, 1)
        score_tile(0, 3)
        proj_qk("k", kT_sb, 0, [1])
        for tt in range(4, TT):
            score_tile(0, tt)
        proj_qk("k", kT_sb, 1, range(TH))
        proj_qk("q", qT_sb, 1, [0])
        scores_group(1)
        proj_qk("k", kT_sb, 2, range(TH))
        proj_qk("q", qT_sb, 2, [0])
        proj_v()
        for ot in range(CT):
            proj_qk("q", qT_sb, ot, [1])
        av_alloc((0, 0), E0)
        scores_group(2, av_plan=((0, 0), 0))
        av_alloc((0, 1), E0)
        scores_group(3, av_plan=((0, 1), 0))

        # lh = 1: lh0's trailing AV group and out_proj(0) fill the AV-free
        # windows of the first two score groups.
        E1 = [[None] * TT for _ in range(SG)]
        state["lh"], state["E"] = 1, E1
        av_alloc((0, 2), E0)
        scores_group(0, av_plan=((0, 2), 0))
        av_alloc((1, 0), E1)
        scores_group(1, av_plan=((1, 0), 2), deferred_tail=True)
        av_alloc((1, 1), E1)
        scores_group(2, av_plan=((1, 1), 2), deferred_tail=True,
                     av_tail=((1, 0), 2))
        # out_proj(0) here: its ring WARs (normalize/recip of av group
        # (1,0)) fired during g2, so the 12 matmuls run stall-free in
        # g2's tail window instead of lengthening the lh-boundary chain
        out_proj(0, use_s4=True)
        av_alloc((1, 2), E1)
        scores_group(3, av_plan=((1, 2), 2), deferred_tail=True,
                     av_tail=((1, 1), 2))
        for tt in range(TT - 2, TT):
            av_sums((1, 2), tt)
        normalize((1, 2))
        out_proj(1, use_s4=True)

    nc.compile()
    return nc


def _prep_inputs(x, conv_w, bn_gamma, bn_beta, bn_mean, bn_var, wq, wk, wv, wo):
    import ml_dtypes

    f32 = np.float32
    bf16 = ml_dtypes.bfloat16
    inv = (bn_gamma / np.sqrt(bn_var + BN_EPS)).astype(f32)
    w9 = (conv_w.reshape(C, 9) * inv[:, None]).astype(f32)
    bias = (bn_beta - bn_mean * inv).astype(f32).reshape(C, 1)
    wqT = np.ascontiguousarray(np.asarray(wq, f32).T).astype(bf16)
    wkT = np.ascontiguousarray(np.asarray(wk, f32).T).astype(bf16)
    wvT = np.ascontiguousarray(np.asarray(wv, f32).T).astype(bf16)
    woT = np.ascontiguousarray(np.asarray(wo, f32).T).astype(bf16)
    ones32 = np.ones((128, 32), f32)
    maps = []
    for b in range(B):
        maps.append(
            {
                "xt": np.ascontiguousarray(np.asarray(x[b], f32).T).astype(bf16),
                "w9": w9,
                "bias": bias,
                "wqT": wqT,
                "wkT": wkT,
                "wvT": wvT,
                "woT": woT,
                "ones32": ones32,
            }
        )
    return maps


def kernel(x, conv_w, bn_gamma, bn_beta, bn_mean, bn_var, wq, wk, wv, wo, h, w,
           **kw):
    assert int(h) == HH and int(w) == WW
    from concourse.bass_utils import run_bass_kernel_spmd

    if "nc" not in _CACHE:
        _CACHE["nc"] = _build()
    nc = _CACHE["nc"]
    maps = _prep_inputs(
        x, conv_w, bn_gamma, bn_beta, bn_mean, bn_var, wq, wk, wv, wo
    )
    res = run_bass_kernel_spmd(nc, maps, list(range(NCORES)))
    out = np.stack([res.results[b]["outT"].T for b in range(B)])
    return out.astype(np.float32)
